# revision 1
# baseline (speedup 1.0000x reference)
"""Trainium2 Bass kernel for multi-head attention graph scatter.

Computes, for each of 8 heads h (one NeuronCore per head):
    q_h = query @ w_q[:, h*32:(h+1)*32]          # [3000, 32]
    k_h = key_emb @ w_k[:, h*32:(h+1)*32]        # [4096, 32]
    attn_h = softmax(q_h @ k_h.T / sqrt(32))     # [3000, 4096]
    graphs[h, qt, :] = attn_h                    # [4096, 4096], rest zeros

kernel(**inputs) takes the full (unsharded) numpy inputs and returns the
full [8, 4096, 4096] float32 output.
"""

import math
import sys

import numpy as np

if "/opt/trn_rl_repo" not in sys.path:
    sys.path.insert(0, "/opt/trn_rl_repo")

N_HEAD = 8
D_K = 32
CONCEPT_NUM = 4096
MASK_NUM = 3000
INPUT_DIM = 256

P = 128  # SBUF partitions
NBLK = 512  # matmul moving-dim tile (one PSUM bank of f32)

_BUILD_CACHE = {}


def _build_module():
    """Build the per-core Bass module (identical on all 8 cores; inputs differ)."""
    import concourse.bacc as bacc
    import concourse.mybir as mybir
    import concourse.tile as tile
    from concourse.masks import make_identity

    f32 = mybir.dt.float32
    f32r = mybir.dt.float32r
    SCALE = 1.0 / math.sqrt(D_K)

    nc = bacc.Bacc("TRN2", target_bir_lowering=False, debug=False, num_devices=N_HEAD)

    query = nc.dram_tensor("query", [MASK_NUM, INPUT_DIM], f32, kind="ExternalInput")
    key_emb = nc.dram_tensor("key_emb", [CONCEPT_NUM, INPUT_DIM], f32, kind="ExternalInput")
    w_qh = nc.dram_tensor("w_qh", [INPUT_DIM, D_K], f32, kind="ExternalInput")
    w_kh = nc.dram_tensor("w_kh", [INPUT_DIM, D_K], f32, kind="ExternalInput")
    graphs = nc.dram_tensor("graphs", [CONCEPT_NUM, CONCEPT_NUM], f32, kind="ExternalOutput")

    # mask-dim tiling: 3000 = 23*128 + 56
    m_tiles = [P] * (MASK_NUM // P) + ([MASK_NUM % P] if MASK_NUM % P else [])
    n_mt = len(m_tiles)
    n_kc = CONCEPT_NUM // NBLK  # 8 concept chunks of 512
    q_chunks = [NBLK] * (MASK_NUM // NBLK) + ([MASK_NUM % NBLK] if MASK_NUM % NBLK else [])
    n_qc = len(q_chunks)  # 6 mask chunks (5x512 + 440)
    n_qt_full = MASK_NUM // P  # 23 full query row-tiles
    mrem = MASK_NUM - n_qt_full * P  # 56

    with tile.TileContext(nc) as tc:
        with (
            tc.tile_pool(name="const", bufs=1) as const_pool,
            tc.tile_pool(name="loads", bufs=6) as loads,
            tc.tile_pool(name="trans", bufs=1) as trans_pool,
            tc.tile_pool(name="proj", bufs=1) as proj_pool,
            tc.tile_pool(name="stats", bufs=4) as stats,
            tc.tile_pool(name="expp", bufs=4) as expp,
            tc.tile_pool(name="tpsum", bufs=3, space="PSUM") as tpsum,
            tc.tile_pool(name="ppsum", bufs=1, space="PSUM") as ppsum,
            tc.tile_pool(name="mpsum", bufs=2, space="PSUM") as mpsum,
        ):
            identity = const_pool.tile([P, P], f32)
            make_identity(nc, identity)

            # warm the PE clock (HAM) before the first real transposes arrive
            for _ in range(8):
                wtp = tpsum.tile([P, 2 * P], f32, tag="tp", name="wtp")
                nc.tensor.transpose(wtp[:, :P], identity[:], identity[:])

            # w slices in lhsT layout: [128, 2, 32] where [p, a, j] = w[a*128+p, j];
            # rounded to f32r for the (f32r) projection matmuls. Tiles declared
            # here; loads/casts are emitted after the first key transposes so
            # they don't occupy the DMA device or DVE queue at t=0.
            wq_f32 = const_pool.tile([P, 2, D_K], f32)
            wk_f32 = const_pool.tile([P, 2, D_K], f32)
            wq_sb = const_pool.tile([P, 2, D_K], f32r)
            wk_sb = const_pool.tile([P, 2, D_K], f32r)

            def emit_w_loads():
                nc.sync.dma_start(wq_f32[:], w_qh.ap().rearrange("(a p) j -> p a j", p=P))
                nc.sync.dma_start(wk_f32[:], w_kh.ap().rearrange("(a p) j -> p a j", p=P))
                nc.vector.tensor_copy(wq_sb[:], wq_f32[:])
                nc.vector.tensor_copy(wk_sb[:], wk_f32[:])

            # transposed input staging (f32r, rounded by the PSUM->SBUF copies)
            keyT = [
                [trans_pool.tile([P, NBLK], f32r, tag=f"keyT{a}_{j}", name=f"keyT{a}_{j}") for j in range(n_kc)]
                for a in range(2)
            ]
            queryT = [
                [trans_pool.tile([P, q_chunks[j]], f32r, tag=f"queryT{a}_{j}", name=f"queryT{a}_{j}") for j in range(n_qc)]
                for a in range(2)
            ]
            kT = [proj_pool.tile([D_K, NBLK], f32r, tag=f"kT_{j}", name=f"kT_{j}") for j in range(n_kc)]
            qT = [proj_pool.tile([D_K, q_chunks[j]], f32r, tag=f"qT_{j}", name=f"qT_{j}") for j in range(n_qc)]

            # ---------- helpers ----------
            copy_flip = [0]

            def transpose_pair(src_a, src_b, dst, col):
                """PE-transpose two [rows<=128, 128] blocks into one PSUM tile,
                then one wide copy into dst[:, col:...]. src_b may be None."""
                tp = tpsum.tile([P, 2 * P], f32, tag="tp", name="tp")
                ra = src_a.shape[0]
                nc.tensor.transpose(tp[:, :ra], src_a, identity[:ra, :ra])
                w = ra
                if src_b is not None:
                    rb = src_b.shape[0]
                    nc.tensor.transpose(tp[:, ra : ra + rb], src_b, identity[:rb, :rb])
                    w += rb
                copy_flip[0] = (copy_flip[0] + 1) % 3
                if copy_flip[0] < 2:
                    nc.vector.tensor_copy(dst[:, col : col + w], tp[:, :w])
                else:
                    nc.scalar.copy(dst[:, col : col + w], tp[:, :w])

            def load_query_group(g):
                """Load query row-tiles 4g..4g+3 (or the 440/56 tail) and transpose."""
                qtile = loads.tile([P, 4, INPUT_DIM], f32, tag="ld", name="qload")
                t0 = g * 4
                t1 = min(t0 + 4, n_qt_full)
                eng = nc.sync if g == 0 else nc.gpsimd
                if t1 > t0:
                    src = query.ap()[t0 * P : t1 * P, :].rearrange("(t p) d -> p t d", p=P)
                    eng.dma_start(qtile[:, : t1 - t0, :], src)
                has_tail = g == 5
                if has_tail:
                    nc.gpsimd.dma_start(qtile[:mrem, 3, :], query.ap()[n_qt_full * P :, :])
                n_full = t1 - t0
                for a in range(2):
                    for tp2 in range(0, n_full - 1, 2):
                        transpose_pair(
                            qtile[:, tp2, a * P : (a + 1) * P],
                            qtile[:, tp2 + 1, a * P : (a + 1) * P],
                            queryT[a][g],
                            tp2 * P,
                        )
                    if n_full % 2:  # odd leftover full tile (group 5: t=2)
                        transpose_pair(
                            qtile[:, n_full - 1, a * P : (a + 1) * P],
                            qtile[:mrem, 3, a * P : (a + 1) * P] if has_tail else None,
                            queryT[a][g],
                            (n_full - 1) * P,
                        )

            def project(dst, srcT, w_sb, width):
                ps = ppsum.tile([D_K, NBLK], f32, tag="pps", name="pps")
                nc.tensor.matmul(ps[:, :width], w_sb[:, 0, :], srcT[0][:], start=True, stop=False)
                nc.tensor.matmul(ps[:, :width], w_sb[:, 1, :], srcT[1][:], start=False, stop=True)
                nc.vector.tensor_copy(dst[:], ps[:, :width])

            def emit_query_chunk(g):
                load_query_group(g)
                project(qT[g], [queryT[0][g], queryT[1][g]], wq_sb, q_chunks[g])

            def scores_chunk(i, ps, j2_off, j):
                """One [mt, 512] scores matmul for m-tile i into psum slice j2_off."""
                mt = m_tiles[i]
                cj = i // 4
                c0 = i * P - cj * NBLK
                nc.tensor.matmul(
                    ps[:mt, j2_off * NBLK : (j2_off + 1) * NBLK],
                    qT[cj][:, c0 : c0 + mt],
                    kT[j][:],
                    start=True,
                    stop=True,
                )

            def exp_chunk(i, ps, exp_dst, sums, h4):
                mt = m_tiles[i]
                nc.scalar.activation(
                    exp_dst[:mt, h4 * 2 * NBLK : (h4 + 1) * 2 * NBLK],
                    ps[:mt, :],
                    mybir.ActivationFunctionType.Exp,
                    scale=SCALE,
                    accum_out=sums[:mt, h4 : h4 + 1],
                )

            def normalize(i, exp_dst, sums, tag):
                mt = m_tiles[i]
                tot = stats.tile([P, 1], f32, tag=f"tot{tag}", name="tot")
                rec = stats.tile([P, 1], f32, tag=f"rec{tag}", name="rec")
                nc.vector.tensor_reduce(
                    tot[:mt], sums[:mt, :], axis=mybir.AxisListType.X, op=mybir.AluOpType.add
                )
                nc.vector.reciprocal(rec[:mt], tot[:mt])
                nc.vector.tensor_scalar_mul(exp_dst[:mt, :], exp_dst[:mt, :], rec[:mt])

            def softmax_tile(i, exp_dst):
                """Full scores+exp+normalize for m-tile i into exp_dst [P, C]."""
                sums = stats.tile([P, 4], f32, tag="sums", name="sums")
                for h4 in range(4):
                    ps = mpsum.tile([P, 2 * NBLK], f32, tag="mps", name="mps")
                    for j2 in range(2):
                        scores_chunk(i, ps, j2, h4 * 2 + j2)
                    exp_chunk(i, ps, exp_dst, sums, h4)
                normalize(i, exp_dst, sums, "")

            def out_dma(i, exp_dst):
                mt = m_tiles[i]
                eng = nc.sync if i % 2 == 0 else nc.gpsimd
                eng.dma_start(graphs.ap()[i * P : i * P + mt, :], exp_dst[:mt, :])

            # ---------- startup: query chunk 0, then key side with tile-0
            # softmax interleaved so the first output DMA starts ASAP ----------
            load_query_group(0)

            key_r = key_emb.ap().rearrange("(t p) d -> p t d", p=P)  # [128, 32, 256]
            exp0 = expp.tile([P, CONCEPT_NUM], f32, tag="exp", name="exp0")
            sums0 = stats.tile([P, 4], f32, tag="sums", name="sums0")
            ps0 = None
            for j in range(n_kc):  # 8 key groups of 4 row-tiles (0.5 MB loads)
                ktile = loads.tile([P, 4, INPUT_DIM], f32, tag="ld", name="kload")
                nc.sync.dma_start(ktile[:], key_r[:, j * 4 : (j + 1) * 4, :])
                for a in range(2):
                    for tp2 in range(0, 4, 2):
                        transpose_pair(
                            ktile[:, tp2, a * P : (a + 1) * P],
                            ktile[:, tp2 + 1, a * P : (a + 1) * P],
                            keyT[a][j],
                            tp2 * P,
                        )
                if j == 0:
                    emit_w_loads()
                    project(qT[0], [queryT[0][0], queryT[1][0]], wq_sb, q_chunks[0])
                project(kT[j], [keyT[0][j], keyT[1][j]], wk_sb, NBLK)
                if j % 2 == 0:
                    ps0 = mpsum.tile([P, 2 * NBLK], f32, tag="mps", name="mps")
                scores_chunk(0, ps0, j % 2, j)
                if j % 2 == 1:
                    exp_chunk(0, ps0, exp0, sums0, j // 2)
            # tile 0: split normalize+DMA into halves so the first HBM write
            # starts as soon as possible (startup latency is the critical path)
            tot0 = stats.tile([P, 1], f32, tag="tot_t0", name="tot0")
            rec0 = stats.tile([P, 1], f32, tag="rec_t0", name="rec0")
            nc.vector.tensor_reduce(
                tot0[:], sums0[:], axis=mybir.AxisListType.X, op=mybir.AluOpType.add
            )
            nc.vector.reciprocal(rec0[:], tot0[:])
            half = CONCEPT_NUM // 2
            nc.vector.tensor_scalar_mul(exp0[:, :half], exp0[:, :half], rec0[:])
            nc.sync.dma_start(graphs.ap()[0:P, :half], exp0[:, :half])
            nc.vector.tensor_scalar_mul(exp0[:, half:], exp0[:, half:], rec0[:])
            nc.sync.dma_start(graphs.ap()[0:P, half:], exp0[:, half:])

            # ---------- main loop; query chunks prefetched one chunk ahead ----------
            done_qc = 1
            for i in range(1, n_mt):
                # prefetch query chunk g one tile before it is needed
                if i % 4 == 3 and done_qc < n_qc and done_qc == (i + 1) // 4:
                    emit_query_chunk(done_qc)
                    done_qc += 1
                exp_t = expp.tile([P, CONCEPT_NUM], f32, tag="exp", name="exp_t")
                softmax_tile(i, exp_t)
                out_dma(i, exp_t)
            while done_qc < n_qc:  # safety (should not trigger)
                emit_query_chunk(done_qc)
                done_qc += 1

    nc.compile()
    return nc


def _get_module():
    if "nc" not in _BUILD_CACHE:
        _BUILD_CACHE["nc"] = _build_module()
    return _BUILD_CACHE["nc"]


def kernel(qt, query, key_emb, w_q, w_k):
    from concourse.bass_utils import run_bass_kernel_spmd

    qt = np.asarray(qt)
    query = np.ascontiguousarray(np.asarray(query, dtype=np.float32))
    key_emb = np.ascontiguousarray(np.asarray(key_emb, dtype=np.float32))
    w_q = np.asarray(w_q, dtype=np.float32)
    w_k = np.asarray(w_k, dtype=np.float32)

    nc = _get_module()
    in_maps = []
    for h in range(N_HEAD):
        in_maps.append(
            {
                "query": query,
                "key_emb": key_emb,
                "w_qh": np.ascontiguousarray(w_q[:, h * D_K : (h + 1) * D_K]),
                "w_kh": np.ascontiguousarray(w_k[:, h * D_K : (h + 1) * D_K]),
            }
        )
    res = run_bass_kernel_spmd(nc, in_maps, core_ids=list(range(N_HEAD)))
    out = np.stack([res.results[h]["graphs"] for h in range(N_HEAD)], axis=0)

    # Device assumes qt == arange(3000) (rows land at graph rows 0..2999,
    # remaining rows stay zero). Remap on host for any other qt.
    if not np.array_equal(qt, np.arange(MASK_NUM)):
        full = np.zeros((N_HEAD, CONCEPT_NUM, CONCEPT_NUM), dtype=np.float32)
        full[:, qt.astype(np.int64), :] = out[:, :MASK_NUM, :]
        out = full
    return out



# revision 5
# speedup vs baseline: 1.2977x; 1.2977x over previous
"""Trainium2 Bass kernel for multi-head attention graph scatter.

Computes, for each of 8 heads h (one NeuronCore per head):
    q_h = query @ w_q[:, h*32:(h+1)*32]          # [3000, 32]
    k_h = key_emb @ w_k[:, h*32:(h+1)*32]        # [4096, 32]
    attn_h = softmax(q_h @ k_h.T / sqrt(32))     # [3000, 4096]
    graphs[h, qt, :] = attn_h                    # [4096, 4096], rest zeros

The device pipeline runs in f16: inputs are host-cast to f16 (padded query
to 3072 rows), loaded pre-transposed via XBAR dma-transpose, projected and
matmul'd in f16, exp'd on the Activation engine straight from PSUM into f16
with the row-sum taken by the activation accumulator, normalized on DVE
(4x mode), and written back as [3000, 4096] f16.  The f32 expansion of the
zero-padded [8, 4096, 4096] output happens on the host.  f16 keeps the
relative error ~5e-4, far inside the 2e-2 gate, and halves HBM traffic.

kernel(**inputs) takes the full (unsharded) numpy inputs and returns the
full [8, 4096, 4096] float32 output.
"""

import math
import sys

import numpy as np

if "/opt/trn_rl_repo" not in sys.path:
    sys.path.insert(0, "/opt/trn_rl_repo")

N_HEAD = 8
D_K = 32
CONCEPT_NUM = 4096
MASK_NUM = 3000
INPUT_DIM = 256

P = 128  # SBUF partitions
MPAD = 3072  # query rows padded to a multiple of 512
NBLK = 512  # matmul moving-dim tile
HALF = 2048  # exp chunk width (4 PSUM banks)

_BUILD_CACHE = {}


def _build_module():
    """Build the per-core Bass module (identical on all 8 cores; inputs differ)."""
    import concourse.bacc as bacc
    import concourse.mybir as mybir
    import concourse.tile as tile

    f32 = mybir.dt.float32
    f16 = mybir.dt.float16
    SCALE = 1.0 / math.sqrt(D_K)

    nc = bacc.Bacc("TRN2", target_bir_lowering=False, debug=False, num_devices=N_HEAD)

    query16 = nc.dram_tensor("query16", [MPAD, INPUT_DIM], f16, kind="ExternalInput")
    key16 = nc.dram_tensor("key16", [CONCEPT_NUM, INPUT_DIM], f16, kind="ExternalInput")
    wq16 = nc.dram_tensor("wq16", [INPUT_DIM, D_K], f16, kind="ExternalInput")
    wk16 = nc.dram_tensor("wk16", [INPUT_DIM, D_K], f16, kind="ExternalInput")
    out16 = nc.dram_tensor("out16", [MASK_NUM, CONCEPT_NUM], f16, kind="ExternalOutput")

    n_mt = MASK_NUM // P + (1 if MASK_NUM % P else 0)  # 24 m-tiles (last is 56 rows)
    n_kc = CONCEPT_NUM // NBLK  # 8 concept chunks of 512
    n_qc = MPAD // NBLK  # 6 query chunks of 512

    with tile.TileContext(nc) as tc:
        with (
            tc.tile_pool(name="const", bufs=1) as const_pool,
            tc.tile_pool(name="trans", bufs=1) as trans_pool,
            tc.tile_pool(name="proj", bufs=1) as proj_pool,
            tc.tile_pool(name="stats", bufs=4) as stats,
            tc.tile_pool(name="expp", bufs=4) as expp,
            tc.tile_pool(name="mpsum", bufs=2, space="PSUM") as mpsum,
        ):
            # ---- PE clock warmup: ~3.5us of dummy matmuls so real matmuls
            # run at full p-state once inputs arrive ----
            warm_op = const_pool.tile([P, 256], f16)
            nc.vector.memset(warm_op[:], 0.0)
            for _ in range(15):
                wps = mpsum.tile([P, 256], f32, tag="mps", name="wps")
                nc.tensor.matmul(
                    wps[:], warm_op[:, :P], warm_op[:], start=True, stop=True
                )

            # ---- weight loads: [128, 2, 32] f16 where [p, a, j] = w[a*128+p, j]
            wq_sb = const_pool.tile([P, 2, D_K], f16)
            wk_sb = const_pool.tile([P, 2, D_K], f16)
            nc.sync.dma_start(wq_sb[:], wq16.ap().rearrange("(a p) j -> p a j", p=P))
            nc.sync.dma_start(wk_sb[:], wk16.ap().rearrange("(a p) j -> p a j", p=P))

            # ---- transposed inputs via XBAR dma-transpose (f16) ----
            queryT = [
                trans_pool.tile([P, MPAD], f16, tag=f"queryT{a}", name=f"queryT{a}")
                for a in range(2)
            ]
            keyT = [
                trans_pool.tile([P, CONCEPT_NUM], f16, tag=f"keyT{a}", name=f"keyT{a}")
                for a in range(2)
            ]
            # query chunk 0 first (the first projection needs it) ...
            for a in range(2):
                nc.sync.dma_start_transpose(
                    queryT[a][:, :NBLK], query16.ap()[:NBLK, a * P : (a + 1) * P]
                )
            # ... then keys in 2048-row chunks (HWDGE issue overhead is ~625ns
            # per DMA, so fewer+bigger transfers win), low concepts first ...
            for c in range(2):
                for a in range(2):
                    nc.sync.dma_start_transpose(
                        keyT[a][:, c * 2048 : (c + 1) * 2048],
                        key16.ap()[c * 2048 : (c + 1) * 2048, a * P : (a + 1) * P],
                    )
            # ... then the rest of the query
            for a in range(2):
                nc.sync.dma_start_transpose(
                    queryT[a][:, NBLK:], query16.ap()[NBLK:, a * P : (a + 1) * P]
                )

            # ---- projections: qT/kT [32, 512] f16 chunks ----
            qT = [proj_pool.tile([D_K, NBLK], f16, tag=f"qT{g}", name=f"qT{g}") for g in range(n_qc)]
            kT = [proj_pool.tile([D_K, NBLK], f16, tag=f"kT{j}", name=f"kT{j}") for j in range(n_kc)]

            def project(dst, srcT, w_sb, c0):
                ps = mpsum.tile([D_K, NBLK], f32, tag="mps", name="pps")
                nc.tensor.matmul(
                    ps[:], w_sb[:, 0, :], srcT[0][:, c0 : c0 + NBLK], start=True, stop=False
                )
                nc.tensor.matmul(
                    ps[:], w_sb[:, 1, :], srcT[1][:, c0 : c0 + NBLK], start=False, stop=True
                )
                nc.vector.tensor_copy(dst[:], ps[:])

            def scores(ps, i, j0, nj, width):
                """nj NBLK-wide score matmuls for m-tile i into psum tile ps."""
                g, c0 = divmod(i * P, NBLK)
                for j in range(j0, j0 + nj):
                    nc.tensor.matmul(
                        ps[:, (j - j0) * NBLK : (j - j0 + 1) * NBLK],
                        qT[g][:, c0 : c0 + P],
                        kT[j][:],
                        start=True,
                        stop=True,
                    )

            def exp_chunk(ps, exp_t, sums, col, c0, width):
                nc.scalar.activation(
                    exp_t[:, c0 : c0 + width],
                    ps[:, :width],
                    mybir.ActivationFunctionType.Exp,
                    scale=SCALE,
                    accum_out=sums[:, col : col + 1],
                )

            def normalize_and_store(i, exp_t, sums, ncols, split=False):
                mt = min(P, MASK_NUM - i * P)
                tot = stats.tile([P, 1], f32, tag="tot", name="tot")
                rec = stats.tile([P, 1], f32, tag="rec", name="rec")
                nc.vector.tensor_reduce(
                    tot[:], sums[:, :ncols], axis=mybir.AxisListType.X, op=mybir.AluOpType.add
                )
                nc.vector.reciprocal(rec[:], tot[:])
                if not split:
                    nc.vector.tensor_scalar_mul(exp_t[:], exp_t[:], rec[:])
                    nc.sync.dma_start(out16.ap()[i * P : i * P + mt, :], exp_t[:mt, :])
                else:  # last tile: pipeline normalize halves into the store
                    for h in range(2):
                        cs = slice(h * HALF, (h + 1) * HALF)
                        nc.vector.tensor_scalar_mul(exp_t[:, cs], exp_t[:, cs], rec[:])
                        nc.sync.dma_start(
                            out16.ap()[i * P : i * P + mt, cs], exp_t[:mt, cs]
                        )

            # ---- tile 0: interleaved with key arrival at 1024 granularity so
            # the Activation engine starts as early as possible ----
            project(qT[0], queryT, wq_sb, 0)
            exp0 = expp.tile([P, CONCEPT_NUM], f16, tag="exp", name="exp0")
            sums0 = stats.tile([P, 4], f32, tag="sums", name="sums0")
            for quarter in range(4):
                project(kT[2 * quarter], keyT, wk_sb, 2 * quarter * NBLK)
                project(kT[2 * quarter + 1], keyT, wk_sb, (2 * quarter + 1) * NBLK)
                ps = mpsum.tile([P, 1024], f32, tag="mps", name="mps0")
                scores(ps, 0, 2 * quarter, 2, 1024)
                exp_chunk(ps, exp0, sums0, quarter, quarter * 1024, 1024)
            normalize_and_store(0, exp0, sums0, 4)

            # ---- main loop: uniform 2048-wide halves ----
            for i in range(1, n_mt):
                g, c0 = divmod(i * P, NBLK)
                exp_t = expp.tile([P, CONCEPT_NUM], f16, tag="exp", name="exp_t")
                sums = stats.tile([P, 2], f32, tag="sums", name="sums")
                for half in range(2):
                    ps = mpsum.tile([P, HALF], f32, tag="mps", name="mps")
                    scores(ps, i, half * 4, 4, HALF)
                    exp_chunk(ps, exp_t, sums, half, half * HALF, HALF)
                    if half == 0 and c0 == 3 * P and g + 1 < n_qc:
                        # prefetch next query chunk's projection mid-tile so the
                        # PSUM-slot disruption hides inside the Act shadow
                        project(qT[g + 1], queryT, wq_sb, (g + 1) * NBLK)
                normalize_and_store(i, exp_t, sums, 2, split=(i == n_mt - 1))

    nc.compile()
    return nc


def _get_module():
    if "nc" not in _BUILD_CACHE:
        _BUILD_CACHE["nc"] = _build_module()
    return _BUILD_CACHE["nc"]


def kernel(qt, query, key_emb, w_q, w_k):
    from concourse.bass_utils import run_bass_kernel_spmd

    qt = np.asarray(qt)
    query16 = np.zeros((MPAD, INPUT_DIM), dtype=np.float16)
    query16[:MASK_NUM] = np.asarray(query, dtype=np.float16)
    key16 = np.ascontiguousarray(np.asarray(key_emb, dtype=np.float16))
    w_q = np.asarray(w_q, dtype=np.float16)
    w_k = np.asarray(w_k, dtype=np.float16)

    nc = _get_module()
    in_maps = []
    for h in range(N_HEAD):
        in_maps.append(
            {
                "query16": query16,
                "key16": key16,
                "wq16": np.ascontiguousarray(w_q[:, h * D_K : (h + 1) * D_K]),
                "wk16": np.ascontiguousarray(w_k[:, h * D_K : (h + 1) * D_K]),
            }
        )
    res = run_bass_kernel_spmd(nc, in_maps, core_ids=list(range(N_HEAD)))
    attn = np.stack([res.results[h]["out16"] for h in range(N_HEAD)], axis=0)

    out = np.zeros((N_HEAD, CONCEPT_NUM, CONCEPT_NUM), dtype=np.float32)
    rows = qt.astype(np.int64) if not np.array_equal(qt, np.arange(MASK_NUM)) else slice(0, MASK_NUM)
    out[:, rows, :] = attn.astype(np.float32)
    return out


# revision 66
# speedup vs baseline: 1.3807x; 1.0639x over previous
"""Trainium2 Bass kernel for multi-head attention graph scatter.

Computes, for each of 8 heads h (one NeuronCore per head):
    q_h = query @ w_q[:, h*32:(h+1)*32]          # [3000, 32]
    k_h = key_emb @ w_k[:, h*32:(h+1)*32]        # [4096, 32]
    attn_h = softmax(q_h @ k_h.T / sqrt(32))     # [3000, 4096]
    graphs[h, qt, :] = attn_h                    # [4096, 4096], rest zeros

The device pipeline runs in f16: inputs are host-cast to f16 (query padded
to 3072 rows with the per-head weights packed into the padding rows), loaded
pre-transposed via XBAR dma-transpose, projected and matmul'd in f16, exp'd
on the Activation engine straight from PSUM into f16 with the row-sum taken
by the activation accumulator (free for PSUM-sourced activations), then
normalized on DVE (4x mode) and written back as [3000, 4096] f16.  The f32
expansion into the zero-padded [8, 4096, 4096] output happens on the host.
f16 keeps the relative error ~7e-4, far inside the 2e-2 gate, and halves
HBM traffic — the modeled bottleneck shifts from DMA (~158us for f32) to
the Activation engine's exp (~102us busy).

kernel(**inputs) takes the full (unsharded) numpy inputs and returns the
full [8, 4096, 4096] float32 output.
"""

import math
import sys

import numpy as np

if "/opt/trn_rl_repo" not in sys.path:
    sys.path.insert(0, "/opt/trn_rl_repo")

N_HEAD = 8
D_K = 32
CONCEPT_NUM = 4096
MASK_NUM = 3000
INPUT_DIM = 256

P = 128  # SBUF partitions
MPAD = 3072  # query rows padded to a multiple of 512
NBLK = 512  # matmul moving-dim tile
HALF = 2048  # exp chunk width (4 PSUM banks)
WQ0 = MASK_NUM + 8  # 3008: first packed w_q row in query16
WK0 = WQ0 + D_K  # 3040: first packed w_k row

_BUILD_CACHE = {}


def _build_module():
    """Build the per-core Bass module (identical on all 8 cores; inputs differ)."""
    import concourse.bacc as bacc
    import concourse.mybir as mybir
    import concourse.tile as tile

    f32 = mybir.dt.float32
    f16 = mybir.dt.float16
    SCALE = 1.0 / math.sqrt(D_K)

    nc = bacc.Bacc("TRN2", target_bir_lowering=False, debug=False, num_devices=N_HEAD)

    # query16 rows: 0-2999 = query, 3000-3007 = zero, 3008-3039 = w_q_head^T,
    # 3040-3071 = w_k_head^T.  Packing the (tiny, per-head) weights into the
    # padded query rows keeps the whole input stream a single homogeneous
    # sequence of XBAR dma-transposes (mixing DMACopy and DmaTranspose on a
    # queue inserts a full completion-wait at each type boundary), and the
    # transpose lands the weights directly in the [partition, d_k] layout the
    # projection matmuls need for lhsT.
    query16 = nc.dram_tensor("query16", [MPAD, INPUT_DIM], f16, kind="ExternalInput")
    key16 = nc.dram_tensor("key16", [CONCEPT_NUM, INPUT_DIM], f16, kind="ExternalInput")
    out16 = nc.dram_tensor("out16", [MASK_NUM, CONCEPT_NUM], f16, kind="ExternalOutput")

    n_mt = MASK_NUM // P + (1 if MASK_NUM % P else 0)  # 24 m-tiles (last is 56 rows)
    n_kc = CONCEPT_NUM // NBLK  # 8 concept chunks of 512
    n_qc = MPAD // NBLK  # 6 query chunks of 512

    with tile.TileContext(nc) as tc:
        with (
            tc.tile_pool(name="trans", bufs=1) as trans_pool,
            tc.tile_pool(name="proj", bufs=1) as proj_pool,
            tc.tile_pool(name="stats", bufs=4) as stats,
            tc.tile_pool(name="expp", bufs=4) as expp,
            tc.tile_pool(name="mpsum", bufs=2, space="PSUM") as mpsum,
        ):
            # ---- transposed inputs via XBAR dma-transpose (f16) ----
            queryT = [
                trans_pool.tile([P, MPAD], f16, tag=f"queryT{a}", name=f"queryT{a}")
                for a in range(2)
            ]
            keyT = [
                trans_pool.tile([P, CONCEPT_NUM], f16, tag=f"keyT{a}", name=f"keyT{a}")
                for a in range(2)
            ]

            def q_rows(r0, r1, eng):
                for a in range(2):
                    eng.dma_start_transpose(
                        queryT[a][:, r0:r1], query16.ap()[r0:r1, a * P : (a + 1) * P]
                    )

            def key_chunk(c):
                for a in range(2):
                    nc.sync.dma_start_transpose(
                        keyT[a][:, c * 1024 : (c + 1) * 1024],
                        key16.ap()[c * 1024 : (c + 1) * 1024, a * P : (a + 1) * P],
                    )

            # Packed weights + query chunk 0 first (tile 0 needs them; the
            # weights ride the Activation HWDGE queue so the sync queue can
            # stream the keys), keys in 1024-row chunks so kT projections
            # proceed in concept order as chunks land, query rest last (not
            # needed until tile 4).  The first 8 transfers are the small
            # critical ones: the scheduler's 8 DMA completion-sem lanes wrap,
            # making transfer #n+8 wait on #n's completion.
            q_rows(WQ0, MPAD, nc.sync)
            q_rows(0, NBLK, nc.sync)
            for c in range(4):
                key_chunk(c)
            q_rows(NBLK, WQ0, nc.sync)

            # ---- projections: qT/kT [32, 512] f16 chunks ----
            qT = [proj_pool.tile([D_K, NBLK], f16, tag=f"qT{g}", name=f"qT{g}") for g in range(n_qc)]
            kT = [proj_pool.tile([D_K, NBLK], f16, tag=f"kT{j}", name=f"kT{j}") for j in range(n_kc)]

            def project(dst, srcT, w0, c0, width=NBLK, use_act=False):
                # qT/kT[d, m] = sum_j w[j, d] * srcT[j, m]; lhsT = the packed
                # weight columns of queryT (w[a*128+p, d] at queryT[a][p, w0+d])
                ps = mpsum.tile([D_K, width], f32, tag="mps", name="pps")
                nc.tensor.matmul(
                    ps[:], queryT[0][:, w0 : w0 + D_K], srcT[0][:, c0 : c0 + width],
                    start=True, stop=False,
                )
                nc.tensor.matmul(
                    ps[:], queryT[1][:, w0 : w0 + D_K], srcT[1][:, c0 : c0 + width],
                    start=False, stop=True,
                )
                if use_act:  # Act engine is idle during startup; its Copy
                    nc.scalar.copy(dst, ps[:])  # unserializes the DVE chain
                else:  # (GPSIMD cannot read PSUM on real hardware)
                    nc.vector.tensor_copy(dst, ps[:])

            def scores(ps, i, j0, nj):
                """nj NBLK-wide score matmuls for m-tile i into psum tile ps."""
                g, c0 = divmod(i * P, NBLK)
                for j in range(j0, j0 + nj):
                    nc.tensor.matmul(
                        ps[:, (j - j0) * NBLK : (j - j0 + 1) * NBLK],
                        qT[g][:, c0 : c0 + P],
                        kT[j][:],
                        start=True,
                        stop=True,
                    )

            def exp_chunk(ps, exp_t, sums, col, c0, width):
                nc.scalar.activation(
                    exp_t[:, c0 : c0 + width],
                    ps[:, :width],
                    mybir.ActivationFunctionType.Exp,
                    scale=SCALE,
                    accum_out=sums[:, col : col + 1],
                )

            def normalize_and_store(i, exp_t, sums, ncols, split=False):
                mt = min(P, MASK_NUM - i * P)
                tot = stats.tile([P, 1], f32, tag="tot", name="tot")
                rec = stats.tile([P, 1], f32, tag="rec", name="rec")
                nc.vector.tensor_reduce(
                    tot[:], sums[:, :ncols], axis=mybir.AxisListType.X, op=mybir.AluOpType.add
                )
                nc.vector.reciprocal(rec[:], tot[:])
                if not split:
                    nc.vector.tensor_scalar_mul(exp_t[:], exp_t[:], rec[:])
                    nc.sync.dma_start(out16.ap()[i * P : i * P + mt, :], exp_t[:mt, :])
                else:  # last tile: pipeline normalize halves into the store
                    for h in range(2):
                        cs = slice(h * HALF, (h + 1) * HALF)
                        nc.vector.tensor_scalar_mul(exp_t[:, cs], exp_t[:, cs], rec[:])
                        nc.sync.dma_start(
                            out16.ap()[i * P : i * P + mt, cs], exp_t[:mt, cs]
                        )

            # ---- tile 0: interleaved with key arrival at 1024 granularity so
            # the Activation engine starts as early as possible ----
            project(qT[0][:], queryT, WQ0, 0)
            exp0 = expp.tile([P, CONCEPT_NUM], f16, tag="exp", name="exp0")
            sums0 = stats.tile([P, 4], f32, tag="sums", name="sums0")
            for quarter in range(4):
                project(kT[2 * quarter][:], keyT, WK0, 2 * quarter * NBLK)
                project(kT[2 * quarter + 1][:], keyT, WK0, (2 * quarter + 1) * NBLK,
                        use_act=True)
                ps = mpsum.tile([P, 1024], f32, tag="mps", name="mps0")
                scores(ps, 0, 2 * quarter, 2)
                exp_chunk(ps, exp0, sums0, quarter, quarter * 1024, 1024)
            normalize_and_store(0, exp0, sums0, 4)

            # ---- main loop: uniform 2048-wide halves ----
            for i in range(1, n_mt):
                g, c0 = divmod(i * P, NBLK)
                if c0 == 3 * P and g + 1 < n_qc:
                    # prefetch the NEXT query chunk's projection one tile ahead
                    # of first use, so tile 4(g+1)'s scores don't wait through
                    # the matmul+copy chain
                    project(qT[g + 1][:], queryT, WQ0, (g + 1) * NBLK)
                exp_t = expp.tile([P, CONCEPT_NUM], f16, tag="exp", name="exp_t")
                sums = stats.tile([P, 2], f32, tag="sums", name="sums")
                for half in range(2):
                    ps = mpsum.tile([P, HALF], f32, tag="mps", name="mps")
                    scores(ps, i, half * 4, 4)
                    exp_chunk(ps, exp_t, sums, half, half * HALF, HALF)
                normalize_and_store(i, exp_t, sums, 2, split=(i == n_mt - 1))

    nc.compile()
    return nc


def _get_module():
    if "nc" not in _BUILD_CACHE:
        _BUILD_CACHE["nc"] = _build_module()
    return _BUILD_CACHE["nc"]


def kernel(qt, query, key_emb, w_q, w_k):
    from concourse.bass_utils import run_bass_kernel_spmd

    qt = np.asarray(qt)
    base = np.zeros((MPAD, INPUT_DIM), dtype=np.float16)
    base[:MASK_NUM] = np.asarray(query, dtype=np.float16)
    key16 = np.ascontiguousarray(np.asarray(key_emb, dtype=np.float16))
    w_q = np.asarray(w_q, dtype=np.float16)
    w_k = np.asarray(w_k, dtype=np.float16)

    nc = _get_module()
    in_maps = []
    for h in range(N_HEAD):
        q16 = base.copy()
        # rows 3008-3039 = w_q_head^T, rows 3040-3071 = w_k_head^T
        q16[WQ0 : WQ0 + D_K] = w_q[:, h * D_K : (h + 1) * D_K].T
        q16[WK0 : WK0 + D_K] = w_k[:, h * D_K : (h + 1) * D_K].T
        in_maps.append({"query16": q16, "key16": key16})
    res = run_bass_kernel_spmd(nc, in_maps, core_ids=list(range(N_HEAD)))
    attn = np.stack([res.results[h]["out16"] for h in range(N_HEAD)], axis=0)

    out = np.zeros((N_HEAD, CONCEPT_NUM, CONCEPT_NUM), dtype=np.float32)
    rows = qt.astype(np.int64) if not np.array_equal(qt, np.arange(MASK_NUM)) else slice(0, MASK_NUM)
    out[:, rows, :] = attn.astype(np.float32)
    return out
